# revision 15
# baseline (speedup 1.0000x reference)
"""Trainium2 Bass kernel for the Kalman graphical-model message-passing problem.

reference math (B=64, D=8, M=4, S=50000):
    m1 = -Qinv @ (xs - F @ x_past)            (B, D, S)
    m2 = FtQinv @ (x_fut - F @ xs)            (B, D, S)
    m3 = HtRinv @ ys_t - (HtRinv @ H) @ xs    (B, D, S)
with x_past/x_fut edge-replicated 1-sample shifts of xs along S.

v2 design (everything bf16 on the wire; rel-err gate is 2e-2, bf16 end-to-end
measures ~7e-3):

  * Algebra: m1 = A1 x_t + B1 x_{t-1} with A1 = -Qinv, B1 = Qinv F.  Then
    m2_t = -F^T m1_{t+1} exactly (including the replicated right edge), so m2
    is ONE matmul over the already-computed m1 tile shifted one column.
    m3 = A3 x_t + C3y y_t with C3 = H^T Rinv, A3 = -(C3 H).
    => 5 matmuls per 512-column chunk instead of 9.

  * Layout: per batch one supertile of NG=16 groups x GW=s/16 samples.
    Partition 8g+j = (group g, state j); columns = samples with one halo
    column each side.  The host PRE-PACKS xs into this exact SBUF image
    (edge replication via clipped gather), and ys transposed into
    partition 4g+m so the ys contraction over m is a single 64-partition
    matmul per chunk (vs 4 stride-4 matmuls).

  * Per batch: 2 load DMAs (x ~800KB, y ~400KB) + 3 store DMAs (~780KB
    each), all with >=6KB contiguous runs.  Per-core traffic ~28.8MB.

  * PSUM fp32, outputs cast to bf16 on the PSUM->SBUF copy (DVE for m1/m2,
    ACT for m3 to split the copy load).
"""

import os
from contextlib import ExitStack

import ml_dtypes
import numpy as np

import concourse.bacc as bacc
import concourse.bass as bass
import concourse.mybir as mybir
import concourse.tile as tile
from concourse.bass_utils import run_bass_kernel_spmd

F32 = mybir.dt.float32
BF16 = mybir.dt.bfloat16
NPBF16 = ml_dtypes.bfloat16

B, D, M, S = 64, 8, 4, 50000
N_CORES = 8
BC = B // N_CORES  # batches per core
NG = 16            # sample groups packed into the 128 partitions
MW = 512           # matmul free-dim / PSUM bank width


def _geom(s):
    assert s % NG == 0, s
    gw = s // NG   # samples per group
    xc = gw + 2    # x cols: 1 past halo + gw + 1 fut halo (cols 0..gw+1 used)
    xc += xc % 2   # pad to even row bytes
    yc = gw + (gw % 2)
    return gw, xc, yc


def _build_nc(bc=BC, s=S):
    variant = os.environ.get("KERNEL_VARIANT", "full")  # perf bisection only
    m2_chain = os.environ.get("KERNEL_M2", "direct") == "chain"
    gw, xc, yc = _geom(s)
    o1w = gw + 1 if m2_chain else gw  # m1 halo col only needed for chaining

    nc = bacc.Bacc(trn_type="TRN2")
    xp = nc.dram_tensor("xp", [bc, 128, xc], BF16, kind="ExternalInput")
    yp = nc.dram_tensor("yp", [bc, 64, yc], BF16, kind="ExternalInput")
    w = nc.dram_tensor("w_all", [128, 7 * 128], BF16, kind="ExternalInput")
    m_all = nc.dram_tensor("m_all", [bc, D, 3, s], BF16, kind="ExternalOutput")

    with tile.TileContext(nc) as tc, ExitStack() as ctx:
        singles = ctx.enter_context(tc.tile_pool(name="singles", bufs=1))
        xpool = ctx.enter_context(tc.tile_pool(name="xp", bufs=3))
        ypool = ctx.enter_context(tc.tile_pool(name="yp", bufs=3))
        o1p = ctx.enter_context(tc.tile_pool(name="o1", bufs=4))
        o2p = ctx.enter_context(tc.tile_pool(name="o2", bufs=4))
        o3p = ctx.enter_context(tc.tile_pool(name="o3", bufs=4))
        pp1 = ctx.enter_context(tc.tile_pool(name="pp1", bufs=3, space="PSUM"))
        pp2 = ctx.enter_context(tc.tile_pool(name="pp2", bufs=2, space="PSUM"))
        pp3 = ctx.enter_context(tc.tile_pool(name="pp3", bufs=3, space="PSUM"))

        w_sb = singles.tile([128, 7 * 128], BF16, tag="w")
        nc.sync.dma_start(out=w_sb[:], in_=w[:, :])
        wA1 = w_sb[:, 0:128]
        wB1 = w_sb[:, 128:256]
        wW2 = w_sb[:, 256:384]
        wA2 = w_sb[:, 384:512]
        wB2 = w_sb[:, 512:640]
        wA3 = w_sb[:, 640:768]
        wC3 = w_sb[0:64, 768:896]

        for b in range(bc):
            xoff = b * 128 * xc
            yoff = b * 64 * yc
            ooff = b * D * 3 * s

            x_t = xpool.tile([128, xc], BF16, tag="x")
            nc.scalar.dma_start(out=x_t[:], in_=bass.AP(xp, xoff, [[xc, 128], [1, xc]]))
            y_t = ypool.tile([64, yc], BF16, tag="y")
            nc.scalar.dma_start(out=y_t[:], in_=bass.AP(yp, yoff, [[yc, 64], [1, yc]]))
            if variant == "loads":
                continue

            o1 = o1p.tile([128, o1w], BF16, tag="o1", name=f"o1_{b}")
            o2 = o2p.tile([128, gw], BF16, tag="o2", name=f"o2_{b}")
            o3 = o3p.tile([128, gw], BF16, tag="o3", name=f"o3_{b}")

            for h0 in range(0, o1w, MW):
                hw1 = min(MW, o1w - h0)
                hw3 = min(MW, gw - h0)
                cur = x_t[:, 1 + h0 : 1 + h0 + hw1]
                past = x_t[:, h0 : h0 + hw1]
                p1 = pp1.tile([128, MW], F32, tag="p1", name=f"p1_{b}_{h0}")
                nc.tensor.matmul(p1[:, 0:hw1], wA1, cur, start=True, stop=False)
                nc.tensor.matmul(p1[:, 0:hw1], wB1, past, start=False, stop=True)
                nc.vector.tensor_copy(out=o1[:, h0 : h0 + hw1], in_=p1[:, 0:hw1])
                if hw3 <= 0:
                    continue
                p3 = pp3.tile([128, MW], F32, tag="p3", name=f"p3_{b}_{h0}")
                nc.tensor.matmul(
                    p3[:, 0:hw3], wA3, cur[:, 0:hw3], start=True, stop=False
                )
                nc.tensor.matmul(
                    p3[:, 0:hw3], wC3, y_t[:, h0 : h0 + hw3], start=False, stop=True
                )
                nc.scalar.copy(out=o3[:, h0 : h0 + hw3], in_=p3[:, 0:hw3])
                if not m2_chain:
                    fut = x_t[:, 2 + h0 : 2 + h0 + hw3]
                    p2 = pp2.tile([128, MW], F32, tag="p2", name=f"p2_{b}_{h0}")
                    nc.tensor.matmul(
                        p2[:, 0:hw3], wA2, cur[:, 0:hw3], start=True, stop=False
                    )
                    nc.tensor.matmul(p2[:, 0:hw3], wB2, fut, start=False, stop=True)
                    nc.vector.tensor_copy(out=o2[:, h0 : h0 + hw3], in_=p2[:, 0:hw3])

            if m2_chain:
                # m2 = -F^T @ m1 shifted one column left
                for h0 in range(0, gw, MW):
                    hw = min(MW, gw - h0)
                    p2 = pp2.tile([128, MW], F32, tag="p2", name=f"p2_{b}_{h0}")
                    nc.tensor.matmul(
                        p2[:, 0:hw],
                        wW2,
                        o1[:, 1 + h0 : 1 + h0 + hw],
                        start=True,
                        stop=True,
                    )
                    nc.vector.tensor_copy(out=o2[:, h0 : h0 + hw], in_=p2[:, 0:hw])

            if variant == "nostores":
                continue
            # stores split across the sync (HWDGE) and gpsimd (SWDGE) rings:
            # each SDMA engine round-robins between rings at packet
            # granularity, so a third stream hides more HBM latency.  o3
            # alternates so both store rings carry ~equal bytes; two column
            # segments let segment A stream while tail chunks compute.
            for o_idx, o_t in ((0, o1), (1, o2), (2, o3)):
                if o_idx == 0:
                    eng = nc.sync
                elif o_idx == 1:
                    eng = nc.gpsimd
                else:
                    eng = nc.sync if b % 2 else nc.gpsimd
                eng.dma_start(
                    out=bass.AP(
                        m_all, ooff + o_idx * s, [[gw, NG], [3 * s, D], [1, gw]]
                    ),
                    in_=o_t[:, 0:gw],
                )
    nc.finalize()
    return nc


def _build_weights(F, H, Q, R):
    """Host-side precompute (init-time work in the torch module)."""
    F64 = np.asarray(F, np.float64)
    H64 = np.asarray(H, np.float64)
    Qinv = np.linalg.inv(np.asarray(Q, np.float64))
    Rinv = np.linalg.inv(np.asarray(R, np.float64))
    A1 = -Qinv
    B1 = Qinv @ F64
    W2 = -F64.T
    C3 = H64.T @ Rinv          # (D, M)
    A3 = -(C3 @ H64)

    A2 = -(F64.T @ Qinv @ F64)
    B2 = F64.T @ Qinv

    eye = np.eye(NG)
    w = np.zeros((128, 7 * 128), NPBF16)
    for i, A in enumerate([A1, B1, W2, A2, B2, A3]):
        # lhsT[8g+j, 8g+i] = A[i, j]  ->  block-diag of A.T
        w[:, i * 128 : (i + 1) * 128] = np.kron(eye, A.T).astype(NPBF16)
    w[0:64, 768:896] = np.kron(eye, C3.T).astype(NPBF16)  # [4g+m, 8g+i] = C3[i, m]
    return w


def _pack_inputs(xs, ys, s):
    """xs (nb, D, s), ys (nb, s, M) f32 -> device images (bf16).

    xp[b, 8g+j, c] = xs[b, j, clip(g*gw + c - 1)]   (c in [0, xc))
    yp[b, 4g+m, c] = ys[b, clip(g*gw + c), m]       (c in [0, yc))
    """
    gw, xc, yc = _geom(s)
    nb = xs.shape[0]
    xs_bf = np.asarray(xs, np.float32).astype(NPBF16)
    g = np.arange(NG)[:, None] * gw
    xcols = np.clip(g + np.arange(xc)[None, :] - 1, 0, s - 1)  # (NG, xc)
    xp = xs_bf[:, :, xcols]                      # (nb, D, NG, xc)
    xp = np.ascontiguousarray(np.swapaxes(xp, 1, 2)).reshape(nb, 128, xc)

    ys_bf = np.swapaxes(np.asarray(ys, np.float32).astype(NPBF16), 1, 2)  # (nb, M, s)
    ycols = np.clip(g + np.arange(yc)[None, :], 0, s - 1)      # (NG, yc)
    yp = ys_bf[:, :, ycols]                      # (nb, M, NG, yc)
    yp = np.ascontiguousarray(np.swapaxes(yp, 1, 2)).reshape(nb, 64, yc)
    return xp, yp


_CACHE = {}


def _get_nc(bc=BC, s=S):
    key = (bc, s)
    if key not in _CACHE:
        _CACHE[key] = _build_nc(bc, s)
    return _CACHE[key]


def run(xs, ys, F, H, Q, R, trace=False, bc=BC, s=S):
    """Shard across 8 cores, run, gather.  Returns ((m1, m2, m3), results)."""
    nb = xs.shape[0]
    assert nb == bc * N_CORES and xs.shape[1:] == (D, s), xs.shape
    assert ys.shape == (nb, s, M), ys.shape
    xp, yp = _pack_inputs(xs, ys, s)
    w_all = _build_weights(F, H, Q, R)

    nc = _get_nc(bc, s)
    in_maps = [
        {
            "xp": np.ascontiguousarray(xp[i * bc : (i + 1) * bc]),
            "yp": np.ascontiguousarray(yp[i * bc : (i + 1) * bc]),
            "w_all": w_all,
        }
        for i in range(N_CORES)
    ]
    res = run_bass_kernel_spmd(nc, in_maps, core_ids=list(range(N_CORES)), trace=trace)
    m_full = np.concatenate([r["m_all"] for r in res.results], axis=0)  # (B,D,3,s) bf16
    outs = tuple(
        np.ascontiguousarray(m_full[:, :, i, :]).astype(np.float32) for i in range(3)
    )
    return outs, res


def kernel(xs, ys, F, H, Q, R):
    trace = bool(int(os.environ.get("KERNEL_TRACE", "0")))
    outs, _ = run(xs, ys, F, H, Q, R, trace=trace)
    return outs
